# revision 29
# baseline (speedup 1.0000x reference)
"""CalderaLinear Trainium2 kernel (fp8 DoubleRow + manual startup weave).

Measured: ~302us HW exec (NTFF) on 8 NeuronCores, rel_absmax err 4.0e-3
(gate 2e-2). Baseline this replaced: 740us.

Computes out = x @ dequant(q).T + (x @ dequant(r).T) @ dequant(l).T + bias
with groupwise (group=128) dequantization, distributed over 8 NeuronCores
by sharding tokens (batch*seq) 8 ways and replicating the weights.

Numerics: the output scale is dominated by the low-rank path (|out| up to
~1.4e6 vs main-path contributions ~2e3), so the x@q.T GEMM runs in fp8
(e4m3) with DoubleRow perf mode (2 fp8 weights/PE cell) at ~2x bf16
throughput while adding negligible absolute error. The low-rank path
(x@r.T and xr@l.T) stays bf16 with fp32 PSUM accumulation. Output is
staged bf16 (abs error well under the gate) and upcast on host.

Host does layout only: dequant-multiply + transpose + fp8/bf16 casts and
token sharding; all matmul work runs on device.

Device per core (1024 tokens):
  x arrives t-tile-major (8 chunks of [128k, 4096] covering 128 tokens
  each) so every chunk unlocks a complete 32-matmul main group. The
  first output-block pair's groups are hand-interleaved with the xr
  (= x @ r_deq.T) phase to keep the PE busy through the DMA-paced
  startup, under an 8-PSUM-bank budget. Engines execute in emission
  order, so emission order here IS the schedule.
"""

import os
import sys

import numpy as np
import ml_dtypes

for _p in ("/opt/trn_rl_repo",):
    if _p not in sys.path and os.path.isdir(_p):
        sys.path.insert(0, _p)

import concourse.bass as bass
import concourse.mybir as mybir
import concourse.tile as tile
from concourse import bacc
from concourse.bass_utils import run_bass_kernel_spmd

BF16 = mybir.dt.bfloat16
F32 = mybir.dt.float32
FP8 = mybir.dt.float8e4
NP_FP8 = ml_dtypes.float8_e4m3
NP_BF16 = ml_dtypes.bfloat16

P = 128  # partitions / dequant group size
N_CORES = 8

# Full problem shape (hardcoded per contest contract).
B, S, D_IN, D_OUT, RANK = 4, 2048, 4096, 4096, 256
N_TOK = B * S          # 8192
T = N_TOK // N_CORES   # 1024 tokens per core
G = D_IN // P          # 32 k-chunks
GP = G // 2            # 16 k-pair-chunks (DoubleRow)
OBW = 512              # output block width
NOB = D_OUT // OBW     # 8 output blocks
RC = RANK // P         # 2 rank chunks
NT = T // P            # 8 token tiles


def caldera_kernel(tc, out, xT_d, q8_d, rT_d, lT_d, biasr_d):
    """One core. DRAM tensors:
    xT_d    [NT, 128, G*128]     bf16  xT[tt,p,(g,tl)] = x[tt*128+tl, g*128+p]
    q8_d    [NOB, 128, GP*2*OBW] fp8   q8[ob,p,(gp,i,o)] =
                                         qdeq[(2gp+i)*128+p, ob*512+o]
    rT_d    [128, G, RANK]       bf16  rT[p,g,r] = rdeq[r, g*128+p]
    lT_d    [128, RC, D_OUT]     bf16  lT[p,c,o] = ldeq[o, c*128+p]
    biasr_d [128, D_OUT]         bf16  bias replicated over partitions
    out     [T, D_OUT]           bf16
    """
    nc = tc.nc
    DR = mybir.MatmulPerfMode.DoubleRow

    with tc.tile_pool(name="const", bufs=1) as constp, \
         tc.tile_pool(name="qsE", bufs=3) as qsE, \
         tc.tile_pool(name="outp", bufs=6) as outp, \
         tc.tile_pool(name="ps", bufs=8, space="PSUM") as psp:

        # ---- resident tensors ----
        x8 = constp.tile([P, G, T], FP8)        # fp8 copy of x.T
        lT = constp.tile([P, RC, D_OUT], BF16)
        biasr = constp.tile([P, D_OUT], BF16)
        xrT = constp.tile([P, RC, T], BF16)     # xr.T chunks

        # HAM warm-up: a chain of dependency-free matmuls long enough to
        # bridge the gap from the NEFF preamble (~7us) to first-data
        # (~15us) with no >3.4us idle window, so the first real matmuls
        # run at the warm 2.4GHz clock instead of 1.2GHz.
        scratch = constp.tile([P, 64], BF16)
        nc.vector.memset(scratch[:], 0.0)
        warm_ps = psp.tile([64, 64], F32, tag="ps", name="warmps")
        for _ in range(135):
            nc.tensor.matmul(
                warm_ps[:], lhsT=scratch[:, 0:64], rhs=scratch[:, 0:64],
                start=True, stop=True,
            )

        qtiles = {}

        def fetch_q(ob, pool, frac=None):
            if ob not in qtiles:
                qtiles[ob] = pool.tile(
                    [P, GP, 2, OBW], FP8, tag="q8b", name=f"q8b{ob}"
                )
            qt = qtiles[ob]
            flat = qt[:].rearrange("p a b c -> p (a b c)")
            if frac is None:
                nc.sync.dma_start(out=flat, in_=q8_d[ob])
            else:
                i, n = frac
                h = GP * 2 * OBW // n
                nc.sync.dma_start(
                    out=flat[:, i * h:(i + 1) * h],
                    in_=q8_d[ob][:, i * h:(i + 1) * h],
                )

        def main_group_open(ps_pair, t, qA, qB, gplo, gphi):
            """DoubleRow MMs for gp in [gplo, gphi); start on gp==0."""
            psA, psB = ps_pair
            for gp in range(gplo, gphi):
                lhs = x8[:, 2 * gp:2 * gp + 2, t * P:(t + 1) * P]
                nc.tensor.matmul(
                    psA[:], lhsT=lhs, rhs=qA[:, gp],
                    start=(gp == 0), stop=False, perf_mode=DR,
                )
                nc.tensor.matmul(
                    psB[:], lhsT=lhs, rhs=qB[:, gp],
                    start=(gp == 0), stop=False, perf_mode=DR,
                )

        def main_group_close(ps_pair, t, obA, obB):
            """Low-rank closers + bias-add copy + store."""
            psA, psB = ps_pair
            for rb in range(RC):
                lhs2 = xrT[:, rb, t * P:(t + 1) * P]
                nc.tensor.matmul(
                    psA[:], lhsT=lhs2,
                    rhs=lT[:, rb, obA * OBW:(obA + 1) * OBW],
                    start=False, stop=(rb == RC - 1),
                )
                nc.tensor.matmul(
                    psB[:], lhsT=lhs2,
                    rhs=lT[:, rb, obB * OBW:(obB + 1) * OBW],
                    start=False, stop=(rb == RC - 1),
                )
            for ps, ob in ((psA, obA), (psB, obB)):
                ot = outp.tile([P, OBW], BF16, tag="ot")
                nc.vector.tensor_tensor(
                    out=ot[:], in0=ps[:],
                    in1=biasr[:, ob * OBW:(ob + 1) * OBW],
                    op=mybir.AluOpType.add,
                )
                nc.sync.dma_start(
                    out=out[t * P:(t + 1) * P, ob * OBW:(ob + 1) * OBW],
                    in_=ot[:],
                )

        def new_pair(name):
            psA = psp.tile([P, OBW], F32, tag="ps", name=f"psA{name}")
            psB = psp.tile([P, OBW], F32, tag="ps", name=f"psB{name}")
            return psA, psB

        # ================= phase 1: startup weave =================
        with tc.tile_pool(name="xphase", bufs=1) as xp:
            # t-tile-major staging: each x tile lands as one contiguous
            # 8KB-per-partition DMA write (strided dst would shatter the
            # transfer into sub-512B descriptors).
            xsb = xp.tile([P, NT, G, P], BF16)
            rT = xp.tile([P, G, RANK], BF16)

            def fetch_x(tt, half=None):
                src = xT_d[tt].rearrange("p (g tl) -> p g tl", g=G)
                if half is None:
                    nc.sync.dma_start(out=xsb[:, tt], in_=src)
                else:
                    h = G // 2
                    nc.sync.dma_start(
                        out=xsb[:, tt, half * h:(half + 1) * h, :],
                        in_=src[:, half * h:(half + 1) * h, :],
                    )

            def cast_x(tt, half=None):
                if half is None:
                    nc.vector.tensor_copy(
                        out=x8[:, :, tt * P:(tt + 1) * P],
                        in_=xsb[:, tt],
                    )
                else:
                    h = G // 2
                    nc.vector.tensor_copy(
                        out=x8[:, half * h:(half + 1) * h,
                               tt * P:(tt + 1) * P],
                        in_=xsb[:, tt, half * h:(half + 1) * h, :],
                    )

            # DMA emission order = fetch priority.
            fetch_x(0)
            fetch_q(0, qsE, frac=(0, 4))
            fetch_q(1, qsE, frac=(0, 4))
            fetch_q(0, qsE, frac=(1, 4))
            fetch_q(1, qsE, frac=(1, 4))
            fetch_x(1)
            fetch_q(0, qsE, frac=(2, 4))
            fetch_q(1, qsE, frac=(2, 4))
            fetch_x(2)
            fetch_q(0, qsE, frac=(3, 4))
            fetch_q(1, qsE, frac=(3, 4))
            nc.sync.dma_start(out=rT[:], in_=rT_d[:])
            fetch_x(3)
            nc.sync.dma_start(out=lT[:], in_=lT_d[:])
            fetch_x(4)
            fetch_x(5)
            fetch_x(6)
            fetch_x(7)
            nc.sync.dma_start(out=biasr[:], in_=biasr_d[:])
            fetch_q(2, qsE)
            for tt in range(NT):
                cast_x(tt)

            qA, qB = qtiles[0], qtiles[1]

            def xr_half(th):
                """xr psum groups (rb in 0..RC) for token half th."""
                pss = [
                    psp.tile([P, OBW], F32, tag="ps", name=f"xrps{th}_{rb}")
                    for rb in range(RC)
                ]
                for g in range(G):
                    for rb in range(RC):
                        nc.tensor.matmul(
                            pss[rb][:],
                            lhsT=rT[:, g, rb * P:(rb + 1) * P],
                            rhs=xsb[:, 4 * th:4 * th + 4, g, :],
                            start=(g == 0),
                            stop=(g == G - 1),
                        )
                for rb in range(RC):
                    nc.scalar.copy(
                        xrT[:, rb, th * OBW:(th + 1) * OBW], pss[rb][:]
                    )

            # PE emission order (= execution order), 8-bank budget:
            pairs = {}
            for t in (0, 1, 2):
                pairs[t] = new_pair(f"0_{t}")
                main_group_open(pairs[t], t, qA, qB, 0, GP)
            xr_half(0)                      # needs x tiles 0-3; 2 banks
            for t in (0, 1, 2):
                main_group_close(pairs[t], t, 0, 1)
            pairs[3] = new_pair("0_3")
            main_group_open(pairs[3], 3, qA, qB, 0, GP)
            main_group_close(pairs[3], 3, 0, 1)
            pairs[4] = new_pair("0_4")
            main_group_open(pairs[4], 4, qA, qB, 0, GP)
            xr_half(1)                      # needs x tiles 4-7; 2 banks
            main_group_close(pairs[4], 4, 0, 1)
            for t in (5, 6, 7):
                pairs[t] = new_pair(f"0_{t}")
                main_group_open(pairs[t], t, qA, qB, 0, GP)
                main_group_close(pairs[t], t, 0, 1)

        # ================= phase 2: remaining block pairs =================
        with tc.tile_pool(name="qsL", bufs=5) as qsL:
            for ob in range(3, NOB):
                fetch_q(ob, qsL)
            for obp in range(1, NOB // 2):
                obA, obB = 2 * obp, 2 * obp + 1
                qA, qB = qtiles[obA], qtiles[obB]
                for t in range(NT):
                    pp = new_pair(f"{obp}_{t}")
                    main_group_open(pp, t, qA, qB, 0, GP)
                    main_group_close(pp, t, obA, obB)


def build_nc():
    nc = bacc.Bacc("TRN2", target_bir_lowering=False, debug=False)
    xT_d = nc.dram_tensor(
        "xT", [NT, P, G * P], BF16, kind="ExternalInput"
    ).ap()
    q8_d = nc.dram_tensor(
        "q8", [NOB, P, GP * 2 * OBW], FP8, kind="ExternalInput"
    ).ap()
    rT_d = nc.dram_tensor("rT", [P, G, RANK], BF16, kind="ExternalInput").ap()
    lT_d = nc.dram_tensor("lT", [P, RC, D_OUT], BF16, kind="ExternalInput").ap()
    biasr_d = nc.dram_tensor(
        "biasr", [P, D_OUT], BF16, kind="ExternalInput"
    ).ap()
    out = nc.dram_tensor("out", [T, D_OUT], BF16, kind="ExternalOutput").ap()
    with tile.TileContext(nc) as tc:
        caldera_kernel(tc, out, xT_d, q8_d, rT_d, lT_d, biasr_d)
    nc.compile()
    return nc


def _dequant(vals, scales):
    rows, cols = vals.shape
    g = cols // P
    v = vals.astype(np.float32).reshape(rows, g, P) * scales[:, :, None]
    return v.reshape(rows, cols)


def make_in_maps(x, q_values, q_scales, l_values, l_scales, r_values, r_scales,
                 bias):
    # q: dequant -> [k, o] transpose -> fp8, packed per 512-col block:
    # q8[ob, p, (gp, i, o)] = qdeq[(2gp+i)*128+p, ob*512+o]
    qdeq = _dequant(np.asarray(q_values), np.asarray(q_scales))  # [o, k]
    qT = np.ascontiguousarray(qdeq.T).astype(NP_FP8)             # [k, o]
    q8 = qT.reshape(GP, 2, P, NOB, OBW).transpose(3, 2, 0, 1, 4)
    q8 = np.ascontiguousarray(q8).reshape(NOB, P, GP * 2 * OBW)

    rdeq = _dequant(np.asarray(r_values), np.asarray(r_scales))  # [r, k]
    rT = np.ascontiguousarray(
        rdeq.T.reshape(G, P, RANK).transpose(1, 0, 2)
    ).astype(NP_BF16)                                            # [p, g, r]

    ldeq = _dequant(np.asarray(l_values), np.asarray(l_scales))  # [o, r]
    lT = np.ascontiguousarray(
        ldeq.T.reshape(RC, P, D_OUT).transpose(1, 0, 2)
    ).astype(NP_BF16)                                            # [p, c, o]

    biasr = np.ascontiguousarray(
        np.broadcast_to(
            np.asarray(bias, dtype=np.float32).astype(NP_BF16), (P, D_OUT)
        )
    )

    xf = np.asarray(x, dtype=np.float32).reshape(N_TOK, D_IN)
    in_maps = []
    for i in range(N_CORES):
        xs = xf[i * T:(i + 1) * T]                               # [t, k]
        # t-tile-major: xT[tt, p, (g, tl)] = x[tt*128+tl, g*128+p]
        xT = np.ascontiguousarray(
            xs.reshape(NT, P, G, P).transpose(0, 3, 2, 1)
        ).astype(NP_BF16).reshape(NT, P, G * P)
        in_maps.append({
            "xT": xT, "q8": q8, "rT": rT, "lT": lT, "biasr": biasr,
        })
    return in_maps


_NC_CACHE = {}


def _get_nc():
    if "nc" not in _NC_CACHE:
        _NC_CACHE["nc"] = build_nc()
    return _NC_CACHE["nc"]


def run(inputs, trace=False, tmpdir=None):
    nc = _get_nc()
    in_maps = make_in_maps(**inputs)
    res = run_bass_kernel_spmd(
        nc, in_maps, list(range(N_CORES)), trace=trace, tmpdir=tmpdir
    )
    shards = [
        np.asarray(res.results[i]["out"]).astype(np.float32)
        for i in range(N_CORES)
    ]
    full = np.concatenate(shards, axis=0).reshape(B, S, D_OUT)
    return full, res


def kernel(**inputs) -> np.ndarray:
    out, _ = run(inputs, trace=False)
    return out
